# revision 20
# baseline (speedup 1.0000x reference)
"""LSH bucketed attention (shared-QK) Trainium2 kernel.

Problem: nn_LSHAttention (B=4, S=8192, D=1024, H=8, BUCKET=64).

Sharding: 8 cores; core c handles batch b=c//2, head-group g=c%2 (4 heads).
Host computes the LSH hash + argsort (exact reference expressions) and
applies each head's sort to x while sharding inputs, so the device receives
x already in per-head sorted order (feature-major). Each core then:
  - projects sorted x to qT/vT per head with the weight block as the
    stationary matmul operand and 512 tokens streaming (amortizes
    LDWEIGHTS; qT comes out feature-major exactly as the scores matmul
    needs it, v is PE-transposed back to token-major),
  - runs bucketed attention per 128-token chunk fused with the projection;
    E = exp(scores) is written into pre-zeroed block-diagonal [128,128]
    tiles so attn@v is a single K=128 matmul per chunk,
  - unsorts o via dma_gather (the only data-dependent DMA on device),
  - computes the partial output projection over its 4 heads, chasing the
    unsort-gather spans.
Host sums the two per-batch partials (tensor-parallel all-reduce host-side).
"""

import sys

sys.path.insert(0, "/opt/trn_rl_repo")

import numpy as np
import ml_dtypes

import concourse.bacc as bacc
import concourse.bass as bass
import concourse.mybir as mybir
import concourse.tile as tile
from concourse.bass_utils import run_bass_kernel_spmd

B, S, D = 4, 8192, 1024
H, DH, BUCKET = 8, 128, 64
HL = 4                 # heads per core
EPS = 1e-4
P = 128
NCHUNK = S // P        # 64 chunks (=2 buckets) per head
NTG = S // 512         # 16 token groups (4 chunks) per head
SCALE = 1.0 / np.sqrt(DH).astype(np.float32)
BF16 = mybir.dt.bfloat16
F32 = mybir.dt.float32
I16 = mybir.dt.int16

_CACHE = {}
GCH = 896            # max gather chunk (desc ring holds ~64 descs/engine; must be %128)


def _gather_spans(total):
    spans, off = [], 0
    while off < total:
        ch = min(GCH, total - off)
        spans.append((off, ch))
        off += ch
    return spans


def build_kernel():
    nc = bacc.Bacc("TRN2", target_bir_lowering=False)

    xsT = nc.dram_tensor("xsT", [HL, D, S], BF16, kind="ExternalInput")
    Wqv = nc.dram_tensor("Wqv", [HL, D, 2 * DH], BF16, kind="ExternalInput")
    bqvT = nc.dram_tensor("bqvT", [HL, 2, DH], F32, kind="ExternalInput")
    Wo = nc.dram_tensor("Wo", [HL * DH, D], BF16, kind="ExternalInput")
    bo = nc.dram_tensor("bo", [D], F32, kind="ExternalInput")
    inv_t = nc.dram_tensor("inv", [HL, P, S // 16], I16, kind="ExternalInput")
    out = nc.dram_tensor("out", [S, D], BF16, kind="ExternalOutput")

    o_dram = nc.dram_tensor("o_dram", [HL, S, DH], BF16)

    Wqv_r = Wqv[:].rearrange("h (o p) f -> p h o f", p=P)      # [128, 4, 8, 256]
    Wo_r = Wo[:].rearrange("(o p) f -> p o f", p=P)            # [128, 4, 1024]
    out_r = out[:].rearrange("(n p) f -> p n f", p=P)          # [128, 64, 1024]

    with tile.TileContext(nc) as tc:
        with (
            tc.tile_pool(name="const", bufs=1) as const,
            tc.tile_pool(name="xin", bufs=3) as xin,
            tc.tile_pool(name="qvst", bufs=3) as qvst,
            tc.tile_pool(name="attn", bufs=2) as attn,
            tc.tile_pool(name="ost", bufs=3) as ost,
            tc.tile_pool(name="ouT", bufs=1) as ouT,
            tc.tile_pool(name="outst", bufs=3) as outst,
            tc.tile_pool(name="mm", bufs=2, space="PSUM") as mm,
            tc.tile_pool(name="up", bufs=2, space="PSUM") as up,
            tc.tile_pool(name="sc", bufs=2, space="PSUM") as sc,
            tc.tile_pool(name="tpu", bufs=2, space="PSUM") as tpu,
        ):
            # ---- resident constants ----
            # wqv on the scalar queue (needed by the very first matmul but
            # must not delay the first x tiles on the sync queue); the rest
            # on the gpsimd queue.
            wqv_sb = const.tile([P, HL, 8, 2 * DH], BF16)
            nc.scalar.dma_start(wqv_sb[:], Wqv_r)
            wo_sb = const.tile([P, HL, D], BF16)
            nc.gpsimd.dma_start(wo_sb[:], Wo_r)
            bqvT_sb = const.tile([P, HL, 2], F32)
            nc.gpsimd.dma_start(bqvT_sb[:], bqvT[:].rearrange("h two p -> p h two"))
            bo_sb = const.tile([P, D], F32)
            nc.gpsimd.dma_start(
                bo_sb[:],
                bass.AP(tensor=bo[:].tensor, offset=bo[:].offset,
                        ap=[[0, P], [1, D]]),
            )
            inv_sb = const.tile([P, HL, S // 16], I16)
            nc.gpsimd.dma_start(inv_sb[:], inv_t[:].rearrange("h p s -> p h s"))
            ident = const.tile([P, P], BF16)
            from concourse import masks
            masks.make_identity(nc, ident[:])
            # pre-zeroed block-diagonal E tiles (ring of 3 across tgs); exp
            # writes only the diagonal 64x64 blocks, off-diagonal stays zero,
            # so attn@v is one K=128 matmul per chunk.
            Ebufs = []
            for j in range(3):
                Eb = const.tile([P, 4, P], BF16, tag=f"E{j}")
                nc.gpsimd.memset(Eb[:], 0.0)
                Ebufs.append(Eb)

            # Three-stage software pipeline over the flattened (head, tg)
            # sequence: proj(i) || transpose+scores(i-1) || attn@v+out(i-2),
            # so the in-order tensor queue never stalls on scalar/DVE results
            # of the same iteration.
            def stage_proj(hl, tg):
                xsT_h = xsT[hl].rearrange("(o p) t -> p o t", p=P)
                x_tile = xin.tile([P, 8, 512], BF16, tag="x")
                nc.sync.dma_start(
                    x_tile[:], xsT_h[:, :, tg * 512:(tg + 1) * 512])
                qT_ps = mm.tile([P, 512], F32, tag="mm")
                for k in range(8):
                    nc.tensor.matmul(qT_ps[:], wqv_sb[:, hl, k, 0:DH],
                                     x_tile[:, k, :],
                                     start=(k == 0), stop=(k == 7))
                qT_sb = qvst.tile([P, 512], BF16, tag="qT")
                nc.vector.tensor_add(
                    qT_sb[:], qT_ps[:],
                    bqvT_sb[:, hl, 0:1].to_broadcast((P, 512)))
                vT_ps = up.tile([P, 512], F32, tag="up")
                for k in range(8):
                    nc.tensor.matmul(vT_ps[:], wqv_sb[:, hl, k, DH:2 * DH],
                                     x_tile[:, k, :],
                                     start=(k == 0), stop=(k == 7))
                # v bias omitted: softmax rows sum to 1, so A@(v+bv) =
                # A@v + bv; the bv term is folded into the host-side
                # output bias (bv @ Wo).
                vT_sb = qvst.tile([P, 512], BF16, tag="vT")
                nc.scalar.copy(vT_sb[:], vT_ps[:])
                return qT_sb, vT_sb

            def stage_scores(i, st):
                qT_sb, vT_sb = st
                v_ps = tpu.tile([P, 512], BF16, tag="tpu")
                for j in range(4):
                    nc.tensor.transpose(
                        v_ps[:, j * P:(j + 1) * P],
                        vT_sb[:, j * P:(j + 1) * P], ident[:])
                v_sb = qvst.tile([P, 4, DH], BF16, tag="v")
                nc.scalar.copy(v_sb[:], v_ps[:])
                scores = sc.tile([P, 256], F32, tag="sc")
                for j in range(4):
                    q0 = qT_sb[:, j * P:j * P + 64]
                    q1 = qT_sb[:, j * P + 64:(j + 1) * P]
                    nc.tensor.matmul(scores[0:64, 64 * j:64 * (j + 1)],
                                     q0, q0, start=True, stop=True)
                    nc.tensor.matmul(scores[64:128, 64 * j:64 * (j + 1)],
                                     q1, q1, start=True, stop=True,
                                     tile_position=(0, 64))
                E_all = Ebufs[i % 3]
                nc.scalar.activation(
                    E_all[0:64, :, 0:64],
                    scores[0:64, :].rearrange("p (c f) -> p c f", c=4),
                    mybir.ActivationFunctionType.Exp,
                    scale=float(SCALE))
                nc.scalar.activation(
                    E_all[64:128, :, 64:128],
                    scores[64:128, :].rearrange("p (c f) -> p c f", c=4),
                    mybir.ActivationFunctionType.Exp,
                    scale=float(SCALE))
                Ssum = attn.tile([P, 4], F32, tag="S")
                R = attn.tile([P, 4], F32, tag="R")
                nc.vector.reduce_sum(Ssum[:], E_all[:],
                                     axis=mybir.AxisListType.X)
                nc.vector.reciprocal(R[:], Ssum[:])
                return E_all, v_sb, R

            def stage_out(hl, tg, st):
                E_all, v_sb, R = st
                U = tpu.tile([P, 512], F32, tag="tpu")
                for j in range(4):
                    nc.tensor.matmul(U[:, j * P:(j + 1) * P],
                                     E_all[:, j, :], v_sb[:, j, :],
                                     start=True, stop=True)
                o_cg = ost.tile([P, 4, DH], BF16, tag="o")
                for j in range(4):
                    if j % 2 == 0:
                        nc.vector.tensor_mul(
                            o_cg[:, j, :], U[:, j * P:(j + 1) * P],
                            R[:, j:j + 1].to_broadcast((P, DH)))
                    else:
                        nc.scalar.mul(
                            o_cg[:, j, :], U[:, j * P:(j + 1) * P],
                            R[:, j:j + 1])
                # scalar queue: o writes must not sit behind prefetched x
                # tiles on the sync queue (the unsort gather waits on them)
                nc.scalar.dma_start(
                    o_dram[hl].rearrange("(n p) f -> p n f", p=P)[
                        :, 4 * tg:4 * (tg + 1), :],
                    o_cg[:])

            o_uT = []

            def issue_gather(hl):
                o_u = ouT.tile([P, 1, S], BF16, tag=f"ouT{hl}", name=f"o_u{hl}")
                for off, ch in _gather_spans(S):
                    nc.gpsimd.dma_gather(
                        o_u[:, :, off:off + ch], o_dram[hl],
                        inv_sb[:, hl, off // 16:(off + ch) // 16],
                        ch, ch, DH, transpose=True)
                o_uT.append(o_u)

            # 3-stage pipeline, flushed at each head boundary so the head's
            # last o writes (and its unsort gather) issue as early as
            # possible instead of trailing into the next head's projections.
            items = [(hl, tg) for hl in range(HL) for tg in range(NTG)]
            s1, s2 = {}, {}
            for i, (hl, tg) in enumerate(items):
                s1[i] = stage_proj(hl, tg)
                if tg == NTG - 1:
                    if (i - 1) in s1:
                        s2[i - 1] = stage_scores(i - 1, s1.pop(i - 1))
                    if (i - 2) in s2:
                        stage_out(*items[i - 2], s2.pop(i - 2))
                    s2[i] = stage_scores(i, s1.pop(i))
                    if (i - 1) in s2:
                        stage_out(*items[i - 1], s2.pop(i - 1))
                    stage_out(hl, tg, s2.pop(i))
                    issue_gather(hl)
                else:
                    if (i - 1) in s1:
                        s2[i - 1] = stage_scores(i - 1, s1.pop(i - 1))
                    if (i - 2) in s2:
                        stage_out(*items[i - 2], s2.pop(i - 2))

            # ---- output projection (partial over 4 heads) ----
            for t in range(NCHUNK):
                ps0 = mm.tile([P, 512], F32, tag="mm")
                ps1 = up.tile([P, 512], F32, tag="up")
                # interleaved so consecutive matmuls share the stationary
                # operand (same o_uT slice for both column halves)
                for hl in range(HL):
                    lhsT = o_uT[hl][:, 0, t * P:(t + 1) * P]
                    nc.tensor.matmul(ps0[:], lhsT, wo_sb[:, hl, 0:512],
                                     start=(hl == 0), stop=(hl == HL - 1))
                    nc.tensor.matmul(ps1[:], lhsT, wo_sb[:, hl, 512:1024],
                                     start=(hl == 0), stop=(hl == HL - 1))
                o_out = outst.tile([P, D], BF16, tag="oo")
                nc.vector.tensor_add(o_out[:, 0:512], ps0[:], bo_sb[:, 0:512])
                nc.vector.tensor_add(o_out[:, 512:1024], ps1[:], bo_sb[:, 512:1024])
                nc.sync.dma_start(out_r[:, t, :], o_out[:])

    nc.compile()
    return nc


def _wrap_idx(perm):
    """int32 [S] -> int16 [128, S//16] wrapped + replicated across 8 Q7 cores."""
    w = perm.astype(np.int16).reshape(S // 16, 16).T       # [16, S//16]
    return np.tile(w, (8, 1))


def _host_prep(x, W_hash):
    """Hash + argsort + per-head sort of x on host CPU, mirroring the
    reference expressions exactly (XLA sort is unsupported on trn2, so any
    reference run in this process necessarily uses the CPU backend ->
    bit-identical angles and ranks)."""
    import jax
    import jax.numpy as jnp

    with jax.default_device(jax.devices("cpu")[0]):
        xj = jnp.asarray(x)
        h = (xj @ jnp.asarray(W_hash)).reshape(B, S, H, 2)
        angles = h[..., 0] / (h[..., 1] + EPS)
        idx = jnp.argsort(angles.transpose(0, 2, 1), axis=2)    # [B, H, S]
        inv = jnp.argsort(idx, axis=2)
        xb = xj.astype(jnp.bfloat16)                             # [B, S, D]
        # x sorted per head, feature-major: [B, H, D, S]
        xs = jnp.take_along_axis(xb[:, None, :, :], idx[..., None], axis=2)
        xsT = jnp.swapaxes(xs, 2, 3)
        return (np.asarray(idx), np.asarray(inv),
                np.asarray(xsT).astype(ml_dtypes.bfloat16))


def kernel(x, W_hash, W_q, b_q, W_v, b_v, W_o, b_o):
    x = np.asarray(x, dtype=np.float32)
    W_q = np.asarray(W_q, dtype=np.float32)
    W_v = np.asarray(W_v, dtype=np.float32)
    W_o = np.asarray(W_o, dtype=np.float32)
    b_q = np.asarray(b_q, dtype=np.float32)
    b_v = np.asarray(b_v, dtype=np.float32)
    b_o = np.asarray(b_o, dtype=np.float32)

    idx, inv, xsT = _host_prep(x, W_hash)

    if "nc" not in _CACHE:
        _CACHE["nc"] = build_kernel()
    nc = _CACHE["nc"]

    bf = ml_dtypes.bfloat16
    in_maps = []
    for c in range(8):
        b, g = c // 2, c % 2
        heads = [4 * g + hl for hl in range(HL)]
        hs = slice(4 * g * DH, (4 * g + HL) * DH)
        # per-head [D, 256] = [Wq_h | Wv_h]
        Wqv_c = np.stack([
            np.concatenate([W_q[:, h * DH:(h + 1) * DH],
                            W_v[:, h * DH:(h + 1) * DH]], axis=1)
            for h in heads]).astype(bf)
        # per-head bias columns [HL, 2, 128]: [h, 0] = b_q head h, [h, 1] = b_v
        bqvT_c = np.stack([
            np.stack([b_q[h * DH:(h + 1) * DH], b_v[h * DH:(h + 1) * DH]])
            for h in heads]).astype(np.float32)
        # v bias is skipped on device (softmax rows sum to 1, so it passes
        # through attention unchanged) -> fold b_v @ W_o into this core's
        # output bias.
        bo_c = (b_o if g == 0 else np.zeros_like(b_o)) + b_v[hs] @ W_o[hs, :]
        inv_c = np.stack([_wrap_idx(inv[b, h]) for h in heads])
        in_maps.append({
            "xsT": np.ascontiguousarray(xsT[b, 4 * g:4 * g + HL]),
            "Wqv": Wqv_c,
            "bqvT": bqvT_c,
            "Wo": W_o[hs, :].astype(bf),
            "bo": bo_c.astype(np.float32),
            "inv": inv_c,
        })

    res = run_bass_kernel_spmd(nc, in_maps, list(range(8)), **_CACHE.get("run_kwargs", {}))
    _CACHE["last_result"] = res

    outp = np.empty((B, S, D), np.float32)
    for b in range(B):
        outp[b] = (res.results[2 * b]["out"].astype(np.float32)
                   + res.results[2 * b + 1]["out"].astype(np.float32))
    return outp


# revision 22
# speedup vs baseline: 1.0208x; 1.0208x over previous
"""LSH bucketed attention (shared-QK) Trainium2 kernel.

Problem: nn_LSHAttention (B=4, S=8192, D=1024, H=8, BUCKET=64).

Sharding: 8 cores; core c handles batch b=c//2, head-group g=c%2 (4 heads).
Host computes the LSH hash + argsort (exact reference expressions) and
applies each head's sort to x while sharding inputs, so the device receives
x already in per-head sorted order (feature-major). Each core then:
  - projects sorted x to qT/vT per head with the weight block as the
    stationary matmul operand and 512 tokens streaming (amortizes
    LDWEIGHTS; qT comes out feature-major exactly as the scores matmul
    needs it, v is PE-transposed back to token-major),
  - runs bucketed attention per 128-token chunk fused with the projection;
    E = exp(scores) is written into pre-zeroed block-diagonal [128,128]
    tiles so attn@v is a single K=128 matmul per chunk,
  - unsorts o via dma_gather (the only data-dependent DMA on device),
  - computes the partial output projection over its 4 heads, chasing the
    unsort-gather spans.
Host sums the two per-batch partials (tensor-parallel all-reduce host-side).
"""

import sys

sys.path.insert(0, "/opt/trn_rl_repo")

import numpy as np
import ml_dtypes

import concourse.bacc as bacc
import concourse.bass as bass
import concourse.mybir as mybir
import concourse.tile as tile
from concourse.bass_utils import run_bass_kernel_spmd

B, S, D = 4, 8192, 1024
H, DH, BUCKET = 8, 128, 64
HL = 4                 # heads per core
EPS = 1e-4
P = 128
NCHUNK = S // P        # 64 chunks (=2 buckets) per head
NTG = S // 512         # 16 token groups (4 chunks) per head
SCALE = 1.0 / np.sqrt(DH).astype(np.float32)
BF16 = mybir.dt.bfloat16
F32 = mybir.dt.float32
I16 = mybir.dt.int16

_CACHE = {}
GCH = 896            # max gather chunk (desc ring holds ~64 descs/engine; must be %128)


def _gather_spans(total):
    spans, off = [], 0
    while off < total:
        ch = min(GCH, total - off)
        spans.append((off, ch))
        off += ch
    return spans


def build_kernel():
    nc = bacc.Bacc("TRN2", target_bir_lowering=False)

    xsT = nc.dram_tensor("xsT", [HL, D, S], BF16, kind="ExternalInput")
    Wqv = nc.dram_tensor("Wqv", [HL, D, 2 * DH], BF16, kind="ExternalInput")
    bqvT = nc.dram_tensor("bqvT", [HL, 2, DH], F32, kind="ExternalInput")
    Wo = nc.dram_tensor("Wo", [HL * DH, D], BF16, kind="ExternalInput")
    bo = nc.dram_tensor("bo", [D], F32, kind="ExternalInput")
    inv_t = nc.dram_tensor("inv", [HL, P, S // 16], I16, kind="ExternalInput")
    out = nc.dram_tensor("out", [S, D], BF16, kind="ExternalOutput")

    o_dram = nc.dram_tensor("o_dram", [HL, S, DH], BF16)

    Wqv_r = Wqv[:].rearrange("h (o p) f -> p h o f", p=P)      # [128, 4, 8, 256]
    Wo_r = Wo[:].rearrange("(o p) f -> p o f", p=P)            # [128, 4, 1024]
    out_r = out[:].rearrange("(n p) f -> p n f", p=P)          # [128, 64, 1024]

    with tile.TileContext(nc) as tc:
        with (
            tc.tile_pool(name="const", bufs=1) as const,
            tc.tile_pool(name="xin", bufs=3) as xin,
            tc.tile_pool(name="qvst", bufs=3) as qvst,
            tc.tile_pool(name="attn", bufs=2) as attn,
            tc.tile_pool(name="ost", bufs=3) as ost,
            tc.tile_pool(name="ouT", bufs=1) as ouT,
            tc.tile_pool(name="outst", bufs=3) as outst,
            tc.tile_pool(name="mm", bufs=2, space="PSUM") as mm,
            tc.tile_pool(name="up", bufs=2, space="PSUM") as up,
            tc.tile_pool(name="sc", bufs=2, space="PSUM") as sc,
            tc.tile_pool(name="tpu", bufs=2, space="PSUM") as tpu,
        ):
            # ---- resident constants ----
            # wqv on the sync queue first (needed by the very first matmul);
            # everything else on the gpsimd queue so x tiles stream without
            # waiting behind const loads.
            wqv_sb = const.tile([P, HL, 8, 2 * DH], BF16)
            nc.sync.dma_start(wqv_sb[:], Wqv_r)
            wo_sb = const.tile([P, HL, D], BF16)
            nc.gpsimd.dma_start(wo_sb[:], Wo_r)
            bqvT_sb = const.tile([P, HL, 2], F32)
            nc.gpsimd.dma_start(bqvT_sb[:], bqvT[:].rearrange("h two p -> p h two"))
            bo_sb = const.tile([P, D], F32)
            nc.gpsimd.dma_start(
                bo_sb[:],
                bass.AP(tensor=bo[:].tensor, offset=bo[:].offset,
                        ap=[[0, P], [1, D]]),
            )
            inv_sb = const.tile([P, HL, S // 16], I16)
            nc.gpsimd.dma_start(inv_sb[:], inv_t[:].rearrange("h p s -> p h s"))
            ident = const.tile([P, P], BF16)
            from concourse import masks
            masks.make_identity(nc, ident[:])
            # pre-zeroed block-diagonal E tiles (ring of 3 across tgs); exp
            # writes only the diagonal 64x64 blocks, off-diagonal stays zero,
            # so attn@v is one K=128 matmul per chunk.
            Ebufs = []
            for j in range(3):
                Eb = const.tile([P, 4, P], BF16, tag=f"E{j}")
                nc.gpsimd.memset(Eb[:], 0.0)
                Ebufs.append(Eb)

            # Three-stage software pipeline over the flattened (head, tg)
            # sequence: proj(i) || transpose+scores(i-1) || attn@v+out(i-2),
            # so the in-order tensor queue never stalls on scalar/DVE results
            # of the same iteration.
            def stage_proj(hl, tg):
                xsT_h = xsT[hl].rearrange("(o p) t -> p o t", p=P)
                x_tile = xin.tile([P, 8, 512], BF16, tag="x")
                nc.sync.dma_start(
                    x_tile[:], xsT_h[:, :, tg * 512:(tg + 1) * 512])
                qT_ps = mm.tile([P, 512], F32, tag="mm")
                for k in range(8):
                    nc.tensor.matmul(qT_ps[:], wqv_sb[:, hl, k, 0:DH],
                                     x_tile[:, k, :],
                                     start=(k == 0), stop=(k == 7))
                qT_sb = qvst.tile([P, 512], BF16, tag="qT")
                nc.vector.tensor_add(
                    qT_sb[:], qT_ps[:],
                    bqvT_sb[:, hl, 0:1].to_broadcast((P, 512)))
                vT_ps = up.tile([P, 512], F32, tag="up")
                for k in range(8):
                    nc.tensor.matmul(vT_ps[:], wqv_sb[:, hl, k, DH:2 * DH],
                                     x_tile[:, k, :],
                                     start=(k == 0), stop=(k == 7))
                # v bias omitted: softmax rows sum to 1, so A@(v+bv) =
                # A@v + bv; the bv term is folded into the host-side
                # output bias (bv @ Wo).
                vT_sb = qvst.tile([P, 512], BF16, tag="vT")
                nc.scalar.copy(vT_sb[:], vT_ps[:])
                return qT_sb, vT_sb

            def stage_scores(i, st):
                qT_sb, vT_sb = st
                v_ps = tpu.tile([P, 512], BF16, tag="tpu")
                for j in range(4):
                    nc.tensor.transpose(
                        v_ps[:, j * P:(j + 1) * P],
                        vT_sb[:, j * P:(j + 1) * P], ident[:])
                v_sb = qvst.tile([P, 4, DH], BF16, tag="v")
                nc.scalar.copy(v_sb[:], v_ps[:])
                scores = sc.tile([P, 256], F32, tag="sc")
                for j in range(4):
                    q0 = qT_sb[:, j * P:j * P + 64]
                    q1 = qT_sb[:, j * P + 64:(j + 1) * P]
                    nc.tensor.matmul(scores[0:64, 64 * j:64 * (j + 1)],
                                     q0, q0, start=True, stop=True)
                    nc.tensor.matmul(scores[64:128, 64 * j:64 * (j + 1)],
                                     q1, q1, start=True, stop=True,
                                     tile_position=(0, 64))
                E_all = Ebufs[i % 3]
                nc.scalar.activation(
                    E_all[0:64, :, 0:64],
                    scores[0:64, :].rearrange("p (c f) -> p c f", c=4),
                    mybir.ActivationFunctionType.Exp,
                    scale=float(SCALE))
                nc.scalar.activation(
                    E_all[64:128, :, 64:128],
                    scores[64:128, :].rearrange("p (c f) -> p c f", c=4),
                    mybir.ActivationFunctionType.Exp,
                    scale=float(SCALE))
                Ssum = attn.tile([P, 4], F32, tag="S")
                R = attn.tile([P, 4], F32, tag="R")
                nc.vector.reduce_sum(Ssum[:], E_all[:],
                                     axis=mybir.AxisListType.X)
                nc.vector.reciprocal(R[:], Ssum[:])
                return E_all, v_sb, R

            def stage_out(hl, tg, st):
                E_all, v_sb, R = st
                U = tpu.tile([P, 512], F32, tag="tpu")
                for j in range(4):
                    nc.tensor.matmul(U[:, j * P:(j + 1) * P],
                                     E_all[:, j, :], v_sb[:, j, :],
                                     start=True, stop=True)
                o_cg = ost.tile([P, 4, DH], BF16, tag="o")
                for j in range(4):
                    if j % 2 == 0:
                        nc.vector.tensor_mul(
                            o_cg[:, j, :], U[:, j * P:(j + 1) * P],
                            R[:, j:j + 1].to_broadcast((P, DH)))
                    else:
                        nc.scalar.mul(
                            o_cg[:, j, :], U[:, j * P:(j + 1) * P],
                            R[:, j:j + 1])
                nc.sync.dma_start(
                    o_dram[hl].rearrange("(n p) f -> p n f", p=P)[
                        :, 4 * tg:4 * (tg + 1), :],
                    o_cg[:])

            o_uT = []

            def issue_gather(hl):
                o_u = ouT.tile([P, 1, S], BF16, tag=f"ouT{hl}", name=f"o_u{hl}")
                for off, ch in _gather_spans(S):
                    nc.gpsimd.dma_gather(
                        o_u[:, :, off:off + ch], o_dram[hl],
                        inv_sb[:, hl, off // 16:(off + ch) // 16],
                        ch, ch, DH, transpose=True)
                o_uT.append(o_u)

            # 3-stage pipeline, flushed at each head boundary so the head's
            # last o writes (and its unsort gather) issue as early as
            # possible instead of trailing into the next head's projections.
            items = [(hl, tg) for hl in range(HL) for tg in range(NTG)]
            s1, s2 = {}, {}
            for i, (hl, tg) in enumerate(items):
                s1[i] = stage_proj(hl, tg)
                if tg == NTG - 1:
                    if (i - 1) in s1:
                        s2[i - 1] = stage_scores(i - 1, s1.pop(i - 1))
                    if (i - 2) in s2:
                        stage_out(*items[i - 2], s2.pop(i - 2))
                    s2[i] = stage_scores(i, s1.pop(i))
                    if (i - 1) in s2:
                        stage_out(*items[i - 1], s2.pop(i - 1))
                    stage_out(hl, tg, s2.pop(i))
                    issue_gather(hl)
                else:
                    if (i - 1) in s1:
                        s2[i - 1] = stage_scores(i - 1, s1.pop(i - 1))
                    if (i - 2) in s2:
                        stage_out(*items[i - 2], s2.pop(i - 2))

            # ---- output projection (partial over 4 heads) ----
            for t in range(NCHUNK):
                ps0 = mm.tile([P, 512], F32, tag="mm")
                ps1 = up.tile([P, 512], F32, tag="up")
                # interleaved so consecutive matmuls share the stationary
                # operand (same o_uT slice for both column halves)
                for hl in range(HL):
                    lhsT = o_uT[hl][:, 0, t * P:(t + 1) * P]
                    nc.tensor.matmul(ps0[:], lhsT, wo_sb[:, hl, 0:512],
                                     start=(hl == 0), stop=(hl == HL - 1))
                    nc.tensor.matmul(ps1[:], lhsT, wo_sb[:, hl, 512:1024],
                                     start=(hl == 0), stop=(hl == HL - 1))
                o_out = outst.tile([P, D], BF16, tag="oo")
                nc.vector.tensor_add(o_out[:, 0:512], ps0[:], bo_sb[:, 0:512])
                nc.vector.tensor_add(o_out[:, 512:1024], ps1[:], bo_sb[:, 512:1024])
                nc.sync.dma_start(out_r[:, t, :], o_out[:])

    nc.compile()
    return nc


def _wrap_idx(perm):
    """int32 [S] -> int16 [128, S//16] wrapped + replicated across 8 Q7 cores."""
    w = perm.astype(np.int16).reshape(S // 16, 16).T       # [16, S//16]
    return np.tile(w, (8, 1))


def _host_prep(x, W_hash):
    """Hash + argsort + per-head sort of x on host CPU, mirroring the
    reference expressions exactly (XLA sort is unsupported on trn2, so any
    reference run in this process necessarily uses the CPU backend ->
    bit-identical angles and ranks)."""
    import jax
    import jax.numpy as jnp

    with jax.default_device(jax.devices("cpu")[0]):
        xj = jnp.asarray(x)
        h = (xj @ jnp.asarray(W_hash)).reshape(B, S, H, 2)
        angles = h[..., 0] / (h[..., 1] + EPS)
        idx = jnp.argsort(angles.transpose(0, 2, 1), axis=2)    # [B, H, S]
        inv = jnp.argsort(idx, axis=2)
        xb = xj.astype(jnp.bfloat16)                             # [B, S, D]
        # x sorted per head, feature-major: [B, H, D, S]
        xs = jnp.take_along_axis(xb[:, None, :, :], idx[..., None], axis=2)
        xsT = jnp.swapaxes(xs, 2, 3)
        return (np.asarray(idx), np.asarray(inv),
                np.asarray(xsT).astype(ml_dtypes.bfloat16))


def kernel(x, W_hash, W_q, b_q, W_v, b_v, W_o, b_o):
    x = np.asarray(x, dtype=np.float32)
    W_q = np.asarray(W_q, dtype=np.float32)
    W_v = np.asarray(W_v, dtype=np.float32)
    W_o = np.asarray(W_o, dtype=np.float32)
    b_q = np.asarray(b_q, dtype=np.float32)
    b_v = np.asarray(b_v, dtype=np.float32)
    b_o = np.asarray(b_o, dtype=np.float32)

    idx, inv, xsT = _host_prep(x, W_hash)

    if "nc" not in _CACHE:
        _CACHE["nc"] = build_kernel()
    nc = _CACHE["nc"]

    bf = ml_dtypes.bfloat16
    in_maps = []
    for c in range(8):
        b, g = c // 2, c % 2
        heads = [4 * g + hl for hl in range(HL)]
        hs = slice(4 * g * DH, (4 * g + HL) * DH)
        # per-head [D, 256] = [Wq_h | Wv_h]
        Wqv_c = np.stack([
            np.concatenate([W_q[:, h * DH:(h + 1) * DH],
                            W_v[:, h * DH:(h + 1) * DH]], axis=1)
            for h in heads]).astype(bf)
        # per-head bias columns [HL, 2, 128]: [h, 0] = b_q head h, [h, 1] = b_v
        bqvT_c = np.stack([
            np.stack([b_q[h * DH:(h + 1) * DH], b_v[h * DH:(h + 1) * DH]])
            for h in heads]).astype(np.float32)
        # v bias is skipped on device (softmax rows sum to 1, so it passes
        # through attention unchanged) -> fold b_v @ W_o into this core's
        # output bias.
        bo_c = (b_o if g == 0 else np.zeros_like(b_o)) + b_v[hs] @ W_o[hs, :]
        inv_c = np.stack([_wrap_idx(inv[b, h]) for h in heads])
        in_maps.append({
            "xsT": np.ascontiguousarray(xsT[b, 4 * g:4 * g + HL]),
            "Wqv": Wqv_c,
            "bqvT": bqvT_c,
            "Wo": W_o[hs, :].astype(bf),
            "bo": bo_c.astype(np.float32),
            "inv": inv_c,
        })

    res = run_bass_kernel_spmd(nc, in_maps, list(range(8)), **_CACHE.get("run_kwargs", {}))
    _CACHE["last_result"] = res

    outp = np.empty((B, S, D), np.float32)
    for b in range(B):
        outp[b] = (res.results[2 * b]["out"].astype(np.float32)
                   + res.results[2 * b + 1]["out"].astype(np.float32))
    return outp
